# revision 1
# baseline (speedup 1.0000x reference)
"""Trainium2 Bass kernel for a 3-layer conditional LSTM (SMILES RNN) with
encoder/decoder feedback.

Math reformulation (verified vs the jax reference):
  - The decoder->encoder feedback path is folded through the rank-47 logits:
      gates0 = A0 @ logits_prev + Wp0 @ props + Whh0 @ h0 + b0c
    with A0 = w_ih0[:, :H] @ enc_w, Wp0 = w_ih0[:, H:], and
    b0c = w_ih0[:, :H] @ enc_b + b_ih0 + b_hh0.  [A0 | Wp0 | b0c] forms one
    K=52 augmented contraction whose stationary operand is
    [logits.T; props.T; ones].
  - t=0 is uniform with logits_init = onehot(1) (the start token encodes to
    exactly enc_w @ onehot1 + enc_b).
  - Logits are produced per-step into an SBUF history buffer and DMA'd out
    once at the end.

Distribution: pure data parallel, batch 128 -> 16 rows per core, weights
replicated; the sequential scan stays core-local (no collectives).

Layout: activations batch-on-partition [16, *]; weights are the *moving*
matmul operand streamed as float32r (full fp32 storage, ~1e-4 matmul
accuracy, 1 cycle/row on TRN2 for moving dim >= 256).  The per-step h must
be transposed ([16,512] -> 4x [128,16]) to serve as the next stationary
operand; done on the PE with an identity matmul.
"""

import numpy as np

B, T, H, O, P, NL = 128, 64, 512, 47, 4, 3
G = 4 * H
NCORES = 8
BL = B // NCORES
KAUG = O + P + 1  # 52
OP = 48  # O padded to even width (fp32r ISA: innermost free count must be even)
MM_DT = "float16"  # matmul operand dtype: "float16" or "float32r"


def _build_nc(t_steps):
    import concourse.mybir as mybir
    import concourse.tile as tile
    from concourse import bacc
    from concourse.masks import make_identity

    F32 = mybir.dt.float32
    F32R = getattr(mybir.dt, MM_DT)
    ACT = mybir.ActivationFunctionType

    nc = bacc.Bacc(None, target_bir_lowering=False)

    w0aug_d = nc.dram_tensor("w0aug", [KAUG, G], F32R, kind="ExternalInput")
    whh0_d = nc.dram_tensor("whh0", [128, 4, G], F32R, kind="ExternalInput")
    w1_d = nc.dram_tensor("w1", [128, 8, G], F32R, kind="ExternalInput")
    w2_d = nc.dram_tensor("w2", [128, 8, G], F32R, kind="ExternalInput")
    dec_d = nc.dram_tensor("dec", [128, 4, OP], F32R, kind="ExternalInput")
    b1_d = nc.dram_tensor("b1", [1, G], F32R, kind="ExternalInput")
    b2_d = nc.dram_tensor("b2", [1, G], F32R, kind="ExternalInput")
    decb_d = nc.dram_tensor("dec_b", [1, OP], F32R, kind="ExternalInput")
    xaug_d = nc.dram_tensor("xaug0", [KAUG, BL], F32R, kind="ExternalInput")
    init_d = nc.dram_tensor("init", [128, NL * 4 * BL + BL], F32R, kind="ExternalInput")
    out_d = nc.dram_tensor("out", [BL, t_steps * O], F32, kind="ExternalOutput")

    with tile.TileContext(nc) as tc:
        with (
            tc.tile_pool(name="weights", bufs=1) as wp,
            tc.tile_pool(name="state", bufs=1) as sp,
            tc.tile_pool(name="htmp", bufs=1) as hp,
            tc.tile_pool(name="gpool", bufs=6 if globals().get("_NCH", 4) == 4 else 3, space="PSUM") as gp,
            tc.tile_pool(name="tpool", bufs=2, space="PSUM") as tp,
        ):
            w0aug = wp.tile([KAUG, G], F32R)
            nc.gpsimd.dma_start(w0aug[:], w0aug_d[:])
            whh0 = wp.tile([128, 4, G], F32R)
            nc.gpsimd.dma_start(whh0[:], whh0_d[:])
            w1 = wp.tile([128, 8, G], F32R)
            nc.gpsimd.dma_start(w1[:], w1_d[:])
            w2 = wp.tile([128, 8, G], F32R)
            nc.gpsimd.dma_start(w2[:], w2_d[:])
            dec = wp.tile([128, 4, OP], F32R)
            nc.gpsimd.dma_start(dec[:], dec_d[:])
            b1 = wp.tile([1, G], F32R)
            nc.gpsimd.dma_start(b1[:], b1_d[:])
            b2 = wp.tile([1, G], F32R)
            nc.gpsimd.dma_start(b2[:], b2_d[:])
            dec_b = wp.tile([1, OP], F32R)
            nc.gpsimd.dma_start(dec_b[:], decb_d[:])

            xaug = sp.tile([KAUG, BL], F32R)
            nc.gpsimd.dma_start(xaug[:], xaug_d[:])
            initt = sp.tile([128, NL * 4 * BL + BL], F32R)
            nc.gpsimd.dma_start(initt[:], init_d[:])
            hT = initt[:, :NL * 4 * BL]
            ones_t = initt[0:1, NL * 4 * BL:NL * 4 * BL + BL]
            ident = sp.tile([BL, BL], F32)
            make_identity(nc, ident)
            cs = []
            for l in range(NL):
                c = sp.tile([BL, H], F32, tag=f"c{l}")
                nc.vector.memset(c[:], 0.0)
                cs.append(c)

            def r(ap):
                return ap

            def hT_sl(l, k):
                j = (l * 4 + k) * BL
                return initt[:, j:j + BL]

            # NCH gate chunks per cell of width GW; narrow (4x512) rotates
            # PSUM slots faster, wide (2x1024) halves matmul issues.
            NCH = globals().get("_NCH", 4)
            GW = G // NCH
            NB = GW * 4 // 2048  # banks per chunk

            def gsl(chunks, lo, hi):
                """yield (global_offset, chunk_ap, slice) covering cols [lo, hi)"""
                for j in range(lo // GW, (hi + GW - 1) // GW):
                    a = max(lo, j * GW) - j * GW
                    b = min(hi, (j + 1) * GW) - j * GW
                    yield j * GW, chunks[j], slice(a, b)

            def emit_hh0(t, ns):
                """cell0 hh matmuls (chunks `ns`) into fresh psum chunks."""
                # (name= explicit: list-comp allocation defeats name inference)
                chunks = [gp.tile([BL, GW], F32, tag="g", name=f"g0_{t}_{n}") for n in ns]
                for chunk, n in zip(chunks, ns):
                    nsl = slice(n * GW, (n + 1) * GW)
                    for k in range(4):
                        nc.tensor.matmul(chunk[:], r(hT_sl(0, k)), r(whh0[:, k, nsl]),
                                         start=(k == 0), stop=False)
                return chunks

            def emit_indep(t, l, wl, bl_t, ns=None):
                chunks = [gp.tile([BL, GW], F32, tag="g", name=f"g{l}_{t}_{n}") for n in (ns or range(NCH))]
                for chunk, n in zip(chunks, ns or range(NCH)):
                    nsl = slice(n * GW, (n + 1) * GW)
                    nc.tensor.matmul(chunk[:], r(ones_t), r(bl_t[:, nsl]),
                                     start=True, stop=False)
                    for k in range(4):
                        nc.tensor.matmul(chunk[:], r(hT_sl(l, k)), r(wl[:, k, nsl]),
                                         start=False, stop=False)
                return chunks

            def emit_inputs(chunks, lsrc, wl):
                for n in range(NCH):
                    nsl = slice(n * GW, (n + 1) * GW)
                    for k in range(4):
                        nc.tensor.matmul(chunks[n][:], r(hT_sl(lsrc, k)), r(wl[:, 4 + k, nsl]),
                                         start=False, stop=(k == 3))

            def lstm_pointwise_transposed(chunks, c, l):
                """Gate nonlinearities + c/h update + h-transposes, half-split
                so the first hT chunks land early for downstream matmuls."""
                ga = hp.tile([BL, G], F32, tag="gact")
                i_ = ga[:, 0 * H:1 * H]
                f_ = ga[:, 1 * H:2 * H]
                g_ = ga[:, 2 * H:3 * H]
                o_ = ga[:, 3 * H:4 * H]
                h = ga[:, 0 * H:1 * H]             # reuse i slot for h
                HH = H // 2
                for off, ch, sl in gsl(chunks, 0, 2 * H):   # sig(i), sig(f)
                    nc.scalar.activation(ga[:, off + sl.start:off + sl.stop], ch[:, sl], ACT.Sigmoid)
                for off, ch, sl in gsl(chunks, 2 * H, 3 * H):
                    nc.scalar.activation(ga[:, off + sl.start:off + sl.stop], ch[:, sl], ACT.Tanh)
                nc.vector.tensor_mul(i_, i_, g_)   # sig(i)*tanh(g)
                nc.vector.tensor_mul(f_, f_, c)    # sig(f)*c
                for off, ch, sl in gsl(chunks, 3 * H, 4 * H):
                    nc.scalar.activation(ga[:, off + sl.start:off + sl.stop], ch[:, sl], ACT.Sigmoid)
                for hf in range(2):
                    sl = slice(hf * HH, (hf + 1) * HH)
                    nc.vector.tensor_add(c[:, sl], i_[:, sl], f_[:, sl])
                    nc.scalar.activation(g_[:, sl], c[:, sl], ACT.Tanh)
                    nc.vector.tensor_mul(h[:, sl], o_[:, sl], g_[:, sl])
                    for k in (2 * hf, 2 * hf + 1):
                        tps = tp.tile([128, BL], F32, tag="tps")
                        nc.tensor.transpose(tps[:], h[:, k * 128:(k + 1) * 128], ident[:])
                        nc.vector.tensor_copy(hT_sl(l, k), tps[:])
                return h

            # prologue: cell0 hh matmuls for t=0
            HALF1 = tuple(range(NCH // 2))
            HALF2 = tuple(range(NCH // 2, NCH))
            g0_chunks = emit_hh0(0, HALF1) + emit_hh0(0, HALF2)
            for t in range(t_steps):
                # (1) cell1 independent: bias + own-h  [dep: hT1(t-1)]
                g1_chunks = emit_indep(t, 1, w1, b1)
                # (2) cell0 aug matmuls  [dep: xaug(t-1 tail)]
                for n in range(NCH):
                    nsl = slice(n * GW, (n + 1) * GW)
                    nc.tensor.matmul(g0_chunks[n][:], r(xaug[:]), r(w0aug[:, nsl]),
                                     start=False, stop=True)
                # (3+5) cell0 pointwise + h0 -> hT0
                lstm_pointwise_transposed(g0_chunks, cs[0], 0)
                # (4a) cell2 independent first half — fills pointwise0
                g2_chunks = emit_indep(t, 2, w2, b2, HALF1)
                # (6) cell1 input matmuls  [dep: hT0(t)]
                emit_inputs(g1_chunks, 0, w1)
                # (4b) cell2 independent second half — fills pointwise1
                g2_chunks = g2_chunks + emit_indep(t, 2, w2, b2, HALF2)
                # (7+9) cell1 pointwise + h1 -> hT1
                lstm_pointwise_transposed(g1_chunks, cs[1], 1)
                # (10) cell2 input matmuls  [dep: hT1(t)]
                emit_inputs(g2_chunks, 1, w2)
                # (11+13) cell2 pointwise + h2 -> hT2
                lstm_pointwise_transposed(g2_chunks, cs[2], 2)
                # (12) next step's cell0 hh (first half) — fills pointwise2
                if t + 1 < t_steps:
                    g0_chunks = emit_hh0(t + 1, HALF1)
                # (14) logits = dec_b + dec @ h2
                lps = tp.tile([BL, OP], F32, tag="tps")
                nc.tensor.matmul(lps[:], r(ones_t), r(dec_b[:]), start=True, stop=False)
                for k in range(4):
                    nc.tensor.matmul(lps[:], r(hT_sl(2, k)), r(dec[:, k, :]),
                                     start=False, stop=(k == 3))
                # (14b) second half of next step's cell0 hh
                if t + 1 < t_steps:
                    g0_chunks = g0_chunks + emit_hh0(t + 1, HALF2)
                # (15) logits tail: out DMA + xaug update
                lt = hp.tile([BL, O], F32, tag="lt")
                nc.vector.tensor_copy(lt[:], lps[:, :O])
                nc.sync.dma_start(out_d[:, t * O:(t + 1) * O], lt[:])
                tps = tp.tile([128, BL], F32, tag="tps")
                nc.tensor.transpose(tps[:O, :], lt[:], ident[:])
                nc.vector.tensor_copy(xaug[0:O, :], tps[:O, :])

    nc.compile()
    return nc


def _init_const():
    init = np.zeros((128, NL * 4 * BL + BL), np.float32)
    init[0, NL * 4 * BL:] = 1.0
    return init


def _host_fold(inputs):
    """Fold encoder/decoder/properties/biases into per-core device inputs."""
    ins = {k: np.asarray(v) for k, v in inputs.items()}
    w_ih0 = ins["w_ih0"].astype(np.float32)
    w_hh0 = ins["w_hh0"].astype(np.float32)
    enc_w = ins["enc_w"].astype(np.float32)
    enc_b = ins["enc_b"].astype(np.float32)
    dec_w = ins["dec_w"].astype(np.float32)
    dec_b = ins["dec_b"].astype(np.float32)
    prop = ins["properties"].astype(np.float32)

    Wx0 = w_ih0[:, :H]
    Wp0 = w_ih0[:, H:]
    A0 = Wx0 @ enc_w                                   # [G, O]
    b0c = Wx0 @ enc_b + ins["b_ih0"] + ins["b_hh0"]    # [G]
    w0aug = np.ascontiguousarray(
        np.concatenate([A0.T, Wp0.T, b0c[None, :].astype(np.float32)], axis=0),
        dtype=np.float32)                              # [52, G]

    def chunked(wT, nk):  # [nk*128, G] -> [128, nk, G]
        return np.ascontiguousarray(
            wT.reshape(nk, 128, wT.shape[1]).transpose(1, 0, 2), dtype=np.float32)

    whh0 = chunked(w_hh0.T, 4)
    W1cat = np.concatenate([ins["w_hh_rest"][0].T, ins["w_ih_rest"][0].T], axis=0)
    W2cat = np.concatenate([ins["w_hh_rest"][1].T, ins["w_ih_rest"][1].T], axis=0)
    w1 = chunked(W1cat.astype(np.float32), 8)
    w2 = chunked(W2cat.astype(np.float32), 8)
    decT_pad = np.zeros((H, OP), np.float32)
    decT_pad[:, :O] = dec_w.T
    dec = chunked(decT_pad, 4)                         # [128, 4, OP]
    b1 = (ins["b_ih_rest"][0] + ins["b_hh_rest"][0]).astype(np.float32)[None, :]
    b2 = (ins["b_ih_rest"][1] + ins["b_hh_rest"][1]).astype(np.float32)[None, :]

    mmdt = np.float16 if MM_DT == "float16" else np.float32
    shared = {
        "w0aug": w0aug.astype(mmdt), "whh0": whh0.astype(mmdt),
        "w1": w1.astype(mmdt), "w2": w2.astype(mmdt), "dec": dec.astype(mmdt),
        "b1": np.ascontiguousarray(b1).astype(mmdt),
        "b2": np.ascontiguousarray(b2).astype(mmdt),
        "dec_b": np.ascontiguousarray(
            np.concatenate([dec_b, np.zeros(OP - O, np.float32)])[None, :]).astype(mmdt),
        "init": _init_const().astype(mmdt),
    }
    in_maps = []
    for cid in range(NCORES):
        xaug = np.zeros((KAUG, BL), np.float32)
        xaug[1, :] = 1.0                               # logits_init = onehot(1)
        xaug[O:O + P, :] = prop[cid * BL:(cid + 1) * BL, :].T
        xaug[O + P, :] = 1.0
        in_maps.append({**shared, "xaug0": np.ascontiguousarray(xaug).astype(mmdt)})
    return in_maps


_NC_CACHE = {}


def _run(inputs, t_steps):
    from concourse.bass_utils import run_bass_kernel_spmd

    if t_steps not in _NC_CACHE:
        _NC_CACHE[t_steps] = _build_nc(t_steps)
    nc = _NC_CACHE[t_steps]
    in_maps = _host_fold(inputs)
    res = run_bass_kernel_spmd(nc, in_maps, core_ids=list(range(NCORES)))
    outs = [res.results[cid]["out"].reshape(BL, t_steps, O) for cid in range(NCORES)]
    return np.concatenate(outs, axis=0)


def kernel(**inputs):
    t_steps = np.asarray(inputs["x"]).shape[1]
    return _run(inputs, t_steps)



# revision 2
# speedup vs baseline: 2052.0705x; 2052.0705x over previous
"""Trainium2 Bass kernel for a 3-layer conditional LSTM (SMILES RNN), v5.

v3a -> v5 (driven by HAM analysis + a width-sweep microbenchmark):
  - The PE clock gate (HAM) throttles to 1.2GHz for any ~3.4us window
    containing PE idle; sustained 4-wide col-tiled streaming stays at
    2.4GHz. The LSTM's pointwise windows (~2us idle, 3x/step) were
    poisoning most windows -> ~80% of matmuls ran at half clock.
    Fix: fill every dependency window with real filler groups first,
    then rotating zero-dependency dummy matmuls (4-strip rotation so
    no WAW stalls) sized to the window.
  - Bias groups moved late in their banks (start=True moves to own-h
    k0) so fillers balance across the three windows.
  - Compact transposes: block-diagonal identity picks only the 64 used
    columns (batch 16 x 4 chunks) -> hT tiles [128, 64], shorter
    transpose stream and copies.

Distribution: pure data parallel, batch 128 -> 16 rows/core, weights
replicated; the scan is core-local (no collectives).
"""

import numpy as np

B, T, H, O, P, NL = 128, 64, 512, 47, 4, 3
G = 4 * H
NCORES = 8
BL = B // NCORES
OP = 48          # O padded
KAUG = 53        # 47 logits + 1 pad + 4 props + 1 ones
KAUGP = 64

# dummy filler matmuls (N=512, ~213ns each) per window: pw0, pw1, pw2, tail
NFILL = (5, 4, 4, 2)


def _build_nc(t_steps):
    import concourse.mybir as mybir
    import concourse.tile as tile
    from concourse import bacc

    F32 = mybir.dt.float32
    F16 = mybir.dt.float16
    ACT = mybir.ActivationFunctionType

    nc = bacc.Bacc(None, target_bir_lowering=False)

    w0aug_d = nc.dram_tensor("w0aug", [KAUGP, G], F16, kind="ExternalInput")
    whh0_d = nc.dram_tensor("whh0", [128, 4, G], F16, kind="ExternalInput")
    w1_d = nc.dram_tensor("w1", [128, 8, G], F16, kind="ExternalInput")
    w2_d = nc.dram_tensor("w2", [128, 8, G], F16, kind="ExternalInput")
    dec_d = nc.dram_tensor("dec", [128, 4, OP], F16, kind="ExternalInput")
    b1_d = nc.dram_tensor("b1", [1, G], F16, kind="ExternalInput")
    b2_d = nc.dram_tensor("b2", [1, G], F16, kind="ExternalInput")
    decb_d = nc.dram_tensor("dec_b", [1, OP], F16, kind="ExternalInput")
    hist0_d = nc.dram_tensor("hist0", [KAUGP, (t_steps + 1) * BL], F16,
                             kind="ExternalInput")
    out_d = nc.dram_tensor("out", [O, t_steps * BL], F16, kind="ExternalOutput")

    with tile.TileContext(nc) as tc:
        with (
            tc.tile_pool(name="weights", bufs=1) as wp,
            tc.tile_pool(name="state", bufs=1) as sp,
            tc.tile_pool(name="work", bufs=2) as hp,
            tc.tile_pool(name="gpool", bufs=1, space="PSUM") as gp,
            tc.tile_pool(name="tpool", bufs=2, space="PSUM") as tp,
            tc.tile_pool(name="lpool", bufs=1, space="PSUM") as lp,
            tc.tile_pool(name="spool", bufs=1, space="PSUM") as scp,
        ):
            w0aug = wp.tile([KAUGP, G], F16)
            nc.gpsimd.dma_start(w0aug[:], w0aug_d[:])
            whh0 = wp.tile([128, 4, G], F16)
            nc.gpsimd.dma_start(whh0[:], whh0_d[:])
            w1 = wp.tile([128, 8, G], F16)
            nc.gpsimd.dma_start(w1[:], w1_d[:])
            w2 = wp.tile([128, 8, G], F16)
            nc.gpsimd.dma_start(w2[:], w2_d[:])
            dec = wp.tile([128, 4, OP], F16)
            nc.gpsimd.dma_start(dec[:], dec_d[:])
            b1 = wp.tile([1, G], F16)
            nc.gpsimd.dma_start(b1[:], b1_d[:])
            b2 = wp.tile([1, G], F16)
            nc.gpsimd.dma_start(b2[:], b2_d[:])
            dec_b = wp.tile([1, OP], F16)
            nc.gpsimd.dma_start(dec_b[:], decb_d[:])

            hist = sp.tile([KAUGP, (t_steps + 1) * BL], F16)
            nc.gpsimd.dma_start(hist[:], hist0_d[:])
            ones = sp.tile([1, BL], F16)
            nc.vector.memset(ones[:], 1.0)

            # block-diagonal identity [128, 4, 32] (uses cols 0:16 of each
            # block): transpose picks only the 64 used hT columns.
            idt = sp.tile([128, 4, 32], F16)
            nc.gpsimd.memset(idt[:], 0.0)
            for j in range(4):
                nc.gpsimd.affine_select(
                    out=idt[:, j, :], in_=idt[:, j, :],
                    compare_op=mybir.AluOpType.not_equal, fill=1.0,
                    base=-32 * j, pattern=[[-1, 32]], channel_multiplier=1)

            hTs, cs = [], []
            for l in range(NL):
                hT = sp.tile([128, 64], F16, tag=f"hT{l}")
                nc.vector.memset(hT[:], 0.0)
                hTs.append(hT)
                c = sp.tile([128, 128], F32, tag=f"c{l}")
                nc.vector.memset(c[:], 0.0)
                cs.append(c)

            scr = scp.tile([128, 512], F32, tag="scr")
            nfill = [0]

            def filler(n):
                """zero-dependency matmuls, 4-strip rotation (no WAW stall):
                keep the PE busy through a dependency window so the HAM
                clock gate never sees an idle window."""
                for _ in range(n):
                    i = nfill[0] = nfill[0] + 1
                    m = i % 4
                    nc.tensor.matmul(scr[32 * m:32 * m + 16, :], ones[:],
                                     b1[:, 0:512], start=True, stop=True,
                                     tile_position=(0, 32 * m))

            def xaug_sl(t):
                return hist[0:KAUG, t * BL:(t + 1) * BL]

            def strips(bank):
                for m in range(4):
                    yield m, bank[32 * m:32 * m + 16, :]

            def emit_own(t, l, bank, w, start):
                """own-h gate matmuls (4 groups); opens bank if start."""
                for k in range(4):
                    for m, o_ in strips(bank):
                        nc.tensor.matmul(o_, hTs[l][:, 16 * k:16 * k + 16],
                                         w[:, k, 512 * m:512 * m + 512],
                                         start=(start and k == 0), stop=False,
                                         tile_position=(0, 32 * m))

            def emit_bias(bank, bias):
                """bias row (1 group, K=1), start=False: joins an open bank."""
                for m, o_ in strips(bank):
                    nc.tensor.matmul(o_, ones[:], bias[:, 512 * m:512 * m + 512],
                                     start=False, stop=False, tile_position=(0, 32 * m))

            def emit_aug0(t, bank):
                """L0 logits/props/bias contraction (1 group), closes bank."""
                for m, o_ in strips(bank):
                    nc.tensor.matmul(o_, xaug_sl(t),
                                     w0aug[0:KAUG, 512 * m:512 * m + 512],
                                     start=False, stop=True, tile_position=(0, 32 * m))

            def emit_inputs(t, l, bank, w):
                """input-h gate matmuls from hT of layer l-1 (4 groups), closes bank."""
                for k in range(4):
                    for m, o_ in strips(bank):
                        nc.tensor.matmul(o_, hTs[l - 1][:, 16 * k:16 * k + 16],
                                         w[:, 4 + k, 512 * m:512 * m + 512],
                                         start=False, stop=(k == 3), tile_position=(0, 32 * m))

            def pointwise(t, bank, l):
                """gates -> c,h update. free layout [i|f|o|g] x128 per strip."""
                ga = hp.tile([128, G // 4], F32, tag="ga", name=f"ga{l}_{t}")
                h = hp.tile([128, 128], F16, tag="h", name=f"h{l}_{t}")
                c = cs[l]
                nc.scalar.activation(ga[:, 0:384], bank[:, 0:384], ACT.Sigmoid)
                nc.scalar.activation(ga[:, 384:512], bank[:, 384:512], ACT.Tanh)
                nc.vector.tensor_mul(c[:], ga[:, 128:256], c[:])                  # f*c
                nc.vector.tensor_mul(ga[:, 0:128], ga[:, 0:128], ga[:, 384:512])  # i*g
                nc.vector.tensor_add(c[:], c[:], ga[:, 0:128])
                nc.scalar.activation(ga[:, 384:512], c[:], ACT.Tanh)
                nc.vector.tensor_mul(h[:], ga[:, 256:384], ga[:, 384:512])        # o*tanh(c)
                return h

            def transpose_h(t, l, h):
                tps = tp.tile([128, 1024], F16, tag="t", name=f"t{l}_{t}")
                nc.tensor.transpose(tps[:, 0:64], h[:], idt[:, :, 0:BL])
                nc.scalar.activation(hTs[l][:], tps[:, 0:64], ACT.Copy)

            def decoder(t):
                """logitsT [48, 16] = dec_w @ h2.T + dec_b; feeds hist slice t+1."""
                lps = lp.tile([OP, 512], F32, tag="lps", name=f"lps_{t}")
                o_ = lps[:, 0:BL]
                nc.tensor.matmul(o_, dec_b[:], ones[:], start=True, stop=False)
                for k in range(4):
                    nc.tensor.matmul(o_, dec[:, k, :], hTs[2][:, 16 * k:16 * k + 16],
                                     start=False, stop=(k == 3))
                nc.vector.tensor_copy(hist[0:OP, (t + 1) * BL:(t + 2) * BL], o_)

            # ---- schedule ----
            g0 = gp.tile([128, G // 4], F32, tag="g0", name="g0_0")
            emit_own(0, 0, g0, whh0, start=True)
            emit_aug0(0, g0)
            g1 = gp.tile([128, G // 4], F32, tag="g1", name="g1_0")
            emit_own(0, 1, g1, w1, start=True)
            emit_bias(g1, b1)
            for t in range(t_steps):
                h0 = pointwise(t, g0, 0)
                g2 = gp.tile([128, G // 4], F32, tag="g2", name=f"g2_{t}")
                emit_own(t, 2, g2, w2, start=True)    # fills pw0 window
                filler(NFILL[0])
                transpose_h(t, 0, h0)
                emit_inputs(t, 1, g1, w1)
                h1 = pointwise(t, g1, 1)
                if t + 1 < t_steps:                   # fills pw1 window
                    g0 = gp.tile([128, G // 4], F32, tag="g0", name=f"g0_{t + 1}")
                    emit_own(t + 1, 0, g0, whh0, start=True)
                emit_bias(g2, b2)
                filler(NFILL[1])
                transpose_h(t, 1, h1)
                emit_inputs(t, 2, g2, w2)
                h2 = pointwise(t, g2, 2)
                if t + 1 < t_steps:                   # fills pw2 window
                    g1 = gp.tile([128, G // 4], F32, tag="g1", name=f"g1_{t + 1}")
                    emit_own(t + 1, 1, g1, w1, start=True)
                    emit_bias(g1, b1)
                filler(NFILL[2])
                transpose_h(t, 2, h2)
                decoder(t)
                filler(NFILL[3])
                if t + 1 < t_steps:
                    emit_aug0(t + 1, g0)              # hist slice t+1 ready
            nc.sync.dma_start(out_d[:], hist[0:O, BL:(t_steps + 1) * BL])

    nc.compile()
    return nc


GATE_PERM = [0, 1, 3, 2]    # torch i,f,g,o -> strip order i,f,o,g


def _strip_cols(Wt):
    """[K, G] (torch gate-major cols i|f|g|o) -> strip-ordered cols:
    col' = 512*m + 128*pos + u, pos order [i, f, o, g]."""
    K = Wt.shape[0]
    W4 = Wt.reshape(K, 4, 4, 128)[:, GATE_PERM]       # [K, Tperm, m, u]
    return np.ascontiguousarray(W4.transpose(0, 2, 1, 3).reshape(K, G))


def _chunk_rows(Wt, nk):
    """[nk*128, N] -> [128, nk, N]"""
    return np.ascontiguousarray(Wt.reshape(nk, 128, Wt.shape[1]).transpose(1, 0, 2))


def _host_fold(inputs, t_steps):
    ins = {k: np.asarray(v) for k, v in inputs.items()}
    f32 = np.float32
    w_ih0 = ins["w_ih0"].astype(f32)
    w_hh0 = ins["w_hh0"].astype(f32)
    enc_w = ins["enc_w"].astype(f32)
    enc_b = ins["enc_b"].astype(f32)
    dec_w = ins["dec_w"].astype(f32)
    dec_b = ins["dec_b"].astype(f32)
    prop = ins["properties"].astype(f32)

    Wx0 = w_ih0[:, :H]
    Wp0 = w_ih0[:, H:]
    A0 = Wx0 @ enc_w                                   # [G, O]
    b0c = Wx0 @ enc_b + ins["b_ih0"] + ins["b_hh0"]    # [G]
    w0aug = np.zeros((KAUGP, G), f32)
    w0aug[0:O, :] = _strip_cols(A0.T)
    w0aug[OP:OP + P, :] = _strip_cols(Wp0.T)
    w0aug[OP + P, :] = _strip_cols(b0c[None, :])[0]

    whh0 = _chunk_rows(_strip_cols(w_hh0.T), 4)
    W1 = np.concatenate([_strip_cols(ins["w_hh_rest"][0].T.astype(f32)),
                         _strip_cols(ins["w_ih_rest"][0].T.astype(f32))], axis=0)
    W2 = np.concatenate([_strip_cols(ins["w_hh_rest"][1].T.astype(f32)),
                         _strip_cols(ins["w_ih_rest"][1].T.astype(f32))], axis=0)
    w1 = _chunk_rows(W1, 8)
    w2 = _chunk_rows(W2, 8)
    decT = np.zeros((H, OP), f32)
    decT[:, :O] = dec_w.T
    dec = _chunk_rows(decT, 4)
    b1 = _strip_cols((ins["b_ih_rest"][0] + ins["b_hh_rest"][0]).astype(f32)[None, :])
    b2 = _strip_cols((ins["b_ih_rest"][1] + ins["b_hh_rest"][1]).astype(f32)[None, :])
    decb = np.concatenate([dec_b, np.zeros(OP - O, f32)])[None, :]

    f16 = np.float16
    shared = {
        "w0aug": w0aug.astype(f16), "whh0": whh0.astype(f16),
        "w1": w1.astype(f16), "w2": w2.astype(f16), "dec": dec.astype(f16),
        "b1": np.ascontiguousarray(b1).astype(f16),
        "b2": np.ascontiguousarray(b2).astype(f16),
        "dec_b": np.ascontiguousarray(decb).astype(f16),
    }
    in_maps = []
    for cid in range(NCORES):
        hist0 = np.zeros((KAUGP, t_steps + 1, BL), f32)
        hist0[1, 0, :] = 1.0                           # logits_init = onehot(1)
        hist0[OP:OP + P, :, :] = prop[cid * BL:(cid + 1) * BL, :].T[:, None, :]
        hist0[OP + P, :, :] = 1.0
        in_maps.append({**shared,
                        "hist0": hist0.reshape(KAUGP, -1).astype(f16)})
    return in_maps


_NC_CACHE = {}


def _run(inputs, t_steps):
    from concourse.bass_utils import run_bass_kernel_spmd

    if t_steps not in _NC_CACHE:
        _NC_CACHE[t_steps] = _build_nc(t_steps)
    nc = _NC_CACHE[t_steps]
    in_maps = _host_fold(inputs, t_steps)
    res = run_bass_kernel_spmd(nc, in_maps, core_ids=list(range(NCORES)))
    outs = [res.results[cid]["out"].astype(np.float32)
            .reshape(O, t_steps, BL).transpose(2, 1, 0) for cid in range(NCORES)]
    return np.concatenate(outs, axis=0)


def kernel(**inputs):
    t_steps = np.asarray(inputs["x"]).shape[1]
    return _run(inputs, t_steps)
